# revision 12
# baseline (speedup 1.0000x reference)
"""Trainium2 Bass kernel for nn_EncoderLayer_54116587929733.

Data-parallel over batch: B=8 batches -> 8 NeuronCores, no collectives.

Per-core math (batch b, S=2048, d=256, H=8 heads, hid=2048), per head h:
    scores = A^T K_h^T / sqrt(d)            [d, S]
    attn   = softmax_S(scores)              [d, S]   (output!)
    M      = attn @ V_h                     [d, d]
    QA     = Q_h @ A                        [S, d]
    ctx    = QA @ M^T                       [S, d]
then concat heads, out-proj + 3 LayerNorms + FFN.

Key restructurings (all exact algebra, no approximation beyond f32r):
  * A is folded into the K/Q projections on the host:
      WKP_h = Wk_h @ A / sqrt(d)  =>  scores = WKP_h^T x^T  (K never built)
      WQP_h = Wq_h @ A            =>  QA^T   = WQP_h^T x^T  (Q never built)
  * V is never materialized:  M^T = Wv_h^T (x^T ETu)  with
    G = x^T ETu computed first (S-contraction), then a tiny d x d matmul.
  * Softmax denominators ride a ones-vector matmul over ETu and are
    folded into the ctx evacuation as a per-partition row scale.
  * Heads are processed in pairs so every big matmul has moving dim 512.
  * Out-proj and FFN2 run transposed ([c, s] output, N=512) and are
    transposed back by PE-transpose blocks fused into the residual add.

All matmuls run in float32r: full fp32 operand width at 1 cycle/row on
the PE for moving dim >= 256 (~1.6e-4 rel err vs 2.6e-3 for bf16).
"""

import sys

sys.path.insert(0, "/opt/trn_rl_repo")

import numpy as np

S = 2048
D = 256
H = 8
B = 8
HID = 2048
EPS = 1e-5
NST = S // 128      # 16 s-tiles
NCT = D // 128      # 2 c/e/dl-tiles
NCH = 4             # s-chunks of 512
CH = S // NCH       # 512

_CACHE = {}


def _split_multi_waits(nc, mybir):
    """This walrus build supports only ONE sync-wait per instruction;
    Tile attaches several.  Move extras onto same-engine NOPs inserted
    just before the offending instruction."""
    cnt = 0
    for fn in nc.m.functions:
        for blk in fn.blocks:
            new_insts = []
            for inst in blk.instructions:
                si = inst.sync_info
                if si is not None and si.on_wait and len(si.on_wait) > 1:
                    waits = list(si.on_wait)
                    for w in waits[:-1]:
                        cnt += 1
                        new_insts.append(mybir.InstNoOp(
                            name=f"{inst.name}_wsplit{cnt}", ins=[], outs=[],
                            engine=inst.engine,
                            sync_info=mybir.SyncInfo(on_wait=[w], on_update=[])))
                    si.on_wait = waits[-1:]
                new_insts.append(inst)
            blk.instructions = new_insts
    return cnt


def _build():
    import concourse.bass as bass
    import concourse.mybir as mybir
    import concourse.tile as tile
    from concourse.masks import make_identity

    f32 = mybir.dt.float32
    f32r = mybir.dt.float32r
    AF = mybir.ActivationFunctionType

    nc = bass.Bass(target_bir_lowering=False)

    # ---- DRAM I/O --------------------------------------------------
    x_d = nc.dram_tensor("x", [S, D], f32r, kind="ExternalInput")
    xT_d = nc.dram_tensor("xT", [D, S], f32r, kind="ExternalInput")
    wkp_d = nc.dram_tensor("wkp", [D, HID], f32r, kind="ExternalInput")
    wqp_d = nc.dram_tensor("wqp", [D, HID], f32r, kind="ExternalInput")
    wv_d = nc.dram_tensor("wv", [D, HID], f32r, kind="ExternalInput")
    wo_d = nc.dram_tensor("wo", [HID, D], f32r, kind="ExternalInput")
    w1_d = nc.dram_tensor("w1", [D, HID], f32r, kind="ExternalInput")
    w2_d = nc.dram_tensor("w2", [HID, D], f32r, kind="ExternalInput")
    b1t_d = nc.dram_tensor("b1t", [128, NST], f32, kind="ExternalInput")
    bvec_names = ["b2b", "g1b", "be1b", "g2b", "be2b", "g3b", "be3b"]
    bvec_d = {n: nc.dram_tensor(n, [128, D], f32, kind="ExternalInput")
              for n in bvec_names}

    attn_d = nc.dram_tensor("attn_o", [H, D, S], f32, kind="ExternalOutput")
    out_d = nc.dram_tensor("out_o", [S, D], f32, kind="ExternalOutput")

    with tile.TileContext(nc) as tc:
        import contextlib
        ctx = contextlib.ExitStack()
        with ctx:
            persist = ctx.enter_context(tc.tile_pool(name="persist", bufs=1))

            xT_s = persist.tile([128, NCT, S], f32r)
            nc.sync.dma_start(out=xT_s[:], in_=xT_d.rearrange("(t p) s -> p t s", p=128))
            x_s = persist.tile([128, NST, D], f32r)
            nc.sync.dma_start(out=x_s[:], in_=x_d.rearrange("(t p) c -> p t c", p=128))
            wqp_s = persist.tile([128, NCT, HID], f32r)
            nc.sync.dma_start(out=wqp_s[:], in_=wqp_d.rearrange("(t p) n -> p t n", p=128))
            wo_s = persist.tile([128, NST, D], f32r)
            nc.sync.dma_start(out=wo_s[:], in_=wo_d.rearrange("(t p) c -> p t c", p=128))
            w1_s = persist.tile([128, NCT, HID], f32r)
            nc.sync.dma_start(out=w1_s[:], in_=w1_d.rearrange("(t p) n -> p t n", p=128))
            w2_s = persist.tile([128, NST, D], f32r)
            nc.sync.dma_start(out=w2_s[:], in_=w2_d.rearrange("(t p) c -> p t c", p=128))
            b1t_s = persist.tile([128, NST], f32)
            nc.sync.dma_start(out=b1t_s[:], in_=b1t_d[:])
            bvec_s = {}
            for n in bvec_names:
                bvec_s[n] = persist.tile([128, D], f32, name=f"bv_{n}", tag=f"bv_{n}")
                nc.sync.dma_start(out=bvec_s[n][:], in_=bvec_d[n][:])

            eps_s = persist.tile([128, 1], f32)
            nc.vector.memset(eps_s, EPS)
            ones_f = persist.tile([128, 1], f32)
            nc.vector.memset(ones_f, 1.0)
            ones_s = persist.tile([128, 1], f32r)
            nc.vector.tensor_copy(out=ones_s[:], in_=ones_f[:])
            ident_s = persist.tile([128, 128], f32)
            make_identity(nc, ident_s)

            # A->B carriers
            mut_s = persist.tile([128, NCT, H, D], f32r)     # [j, jt, h, dl]
            recip_s = persist.tile([128, H, NCT], f32)       # [dl, h, dlt]

            # ------------- Phase A: K/V side, head PAIRS -------------
            with (
                tc.tile_pool(name="poolA", bufs=1) as poolA,
                tc.tile_pool(name="psA", bufs=1, space="PSUM") as psA,
            ):
                wkp_s = poolA.tile([128, NCT, HID], f32r, tag="wkp")
                nc.sync.dma_start(out=wkp_s[:], in_=wkp_d.rearrange("(t p) n -> p t n", p=128))
                wv_s = poolA.tile([128, NCT, HID], f32r, tag="wv")
                nc.sync.dma_start(out=wv_s[:], in_=wv_d.rearrange("(t p) n -> p t n", p=128))

                for pr in range(H // 2):
                    h0 = 2 * pr
                    ps0 = h0 * D                 # pair's column offset in wkp
                    # scores^T for the PAIR: ETu [s, 512] = exp(x WKP_pair)
                    etu_s = poolA.tile([128, NST, 2 * D], f32r, tag="etu", bufs=1)
                    for st in range(NST):
                        stp = psA.tile([128, 2 * D], f32, tag="stp", bufs=2)
                        for kt in range(NCT):
                            nc.tensor.matmul(
                                stp[:], xT_s[:, kt, st * 128:(st + 1) * 128],
                                wkp_s[:, kt, ps0:ps0 + 2 * D],
                                start=(kt == 0), stop=(kt == NCT - 1))
                        nc.scalar.activation(out=etu_s[:, st, :], in_=stp[:],
                                             func=AF.Exp)

                    # denominators for the pair: [1, 512]
                    pd = psA.tile([1, 2 * D], f32, tag="pdx", bufs=1)
                    for st in range(NST):
                        nc.tensor.matmul(pd[:], ones_s[:, :1], etu_s[:, st, :],
                                         start=(st == 0), stop=(st == NST - 1))
                    denr = poolA.tile([1, 2 * D], f32, tag="denr", bufs=1)
                    nc.vector.tensor_copy(out=denr[:], in_=pd[:])
                    for q in range(4):           # q = hh*2 + et
                        hh, et = divmod(q, 2)
                        pdt = psA.tile([128, 1], f32, tag="pdx", bufs=1)
                        nc.tensor.matmul(pdt[:], denr[:1, q * 128:(q + 1) * 128],
                                         ones_f[:1, :1], start=True, stop=True)
                        nc.vector.reciprocal(out=recip_s[:, h0 + hh, et:et + 1],
                                             in_=pdt[:])

                    # G = x^T @ ETu  [c, 512]  (the V-side contraction over S)
                    g_s = poolA.tile([128, NCT, 2 * D], f32r, tag="g2", bufs=1)
                    for ct in range(NCT):
                        pg = psA.tile([128, 2 * D], f32, tag="pg", bufs=2)
                        for st in range(NST):
                            nc.tensor.matmul(
                                pg[:], x_s[:, st, ct * 128:(ct + 1) * 128],
                                etu_s[:, st, :],
                                start=(st == 0), stop=(st == NST - 1))
                        nc.vector.tensor_copy(out=g_s[:, ct, :], in_=pg[:])

                    # MuT_h = Wv_h^T @ G_h   [j, dl]  (tiny d x d matmuls)
                    for hh in range(2):
                        h = h0 + hh
                        for jt in range(NCT):
                            pmu = psA.tile([128, D], f32, tag="pmu", bufs=1)
                            for ct in range(NCT):
                                nc.tensor.matmul(
                                    pmu[:],
                                    wv_s[:, ct, h * D + jt * 128:h * D + (jt + 1) * 128],
                                    g_s[:, ct, hh * D:(hh + 1) * D],
                                    start=(ct == 0), stop=(ct == NCT - 1))
                            nc.vector.tensor_copy(out=mut_s[:, jt, h, :], in_=pmu[:])

                    # scores [e, s] per head -> E -> normalize -> attn out
                    for hh in range(2):
                        h = h0 + hh
                        hs = h * D
                        e_s = poolA.tile([128, NCT, S], f32, tag="e", bufs=1)
                        for et in range(NCT):
                            for chk in range(NCH):
                                sc = psA.tile([128, CH], f32, tag="sc", bufs=2)
                                for kt in range(NCT):
                                    nc.tensor.matmul(
                                        sc[:],
                                        wkp_s[:, kt, hs + et * 128:hs + (et + 1) * 128],
                                        xT_s[:, kt, chk * CH:(chk + 1) * CH],
                                        start=(kt == 0), stop=(kt == NCT - 1))
                                nc.scalar.activation(
                                    out=e_s[:, et, chk * CH:(chk + 1) * CH],
                                    in_=sc[:], func=AF.Exp)
                        for et in range(NCT):
                            nc.gpsimd.tensor_scalar_mul(
                                out=e_s[:, et, :], in0=e_s[:, et, :],
                                scalar1=recip_s[:, h, et:et + 1])
                            nc.sync.dma_start(
                                out=attn_d[h, et * 128:(et + 1) * 128, :],
                                in_=e_s[:, et, :])

            # ---------- Phase B: Q side + out-proj + FFN, per s-chunk ----------
            def ln_block(pool, src_f32_ap, gname, bname, dst_ap):
                """dst = LN(src) * g + b   (src: [128, D] f32 SBUF)"""
                stats = pool.tile([128, 6], f32, tag="lnstats", bufs=4)
                nc.vector.bn_stats(out=stats[:], in_=src_f32_ap)
                mv = pool.tile([128, 2], f32, tag="lnmv", bufs=4)
                nc.vector.bn_aggr(out=mv[:], in_=stats[:])
                nc.scalar.activation(out=mv[:, 1:2], in_=mv[:, 1:2],
                                     func=AF.Sqrt, bias=eps_s[:], scale=1.0)
                nc.vector.reciprocal(out=mv[:, 1:2], in_=mv[:, 1:2])
                nc.vector.tensor_scalar(
                    out=dst_ap, in0=src_f32_ap,
                    scalar1=mv[:, 0:1], scalar2=mv[:, 1:2],
                    op0=mybir.AluOpType.subtract, op1=mybir.AluOpType.mult)
                nc.vector.tensor_mul(out=dst_ap, in0=dst_ap, in1=bvec_s[gname][:])
                nc.vector.tensor_add(out=dst_ap, in0=dst_ap, in1=bvec_s[bname][:])

            with (
                tc.tile_pool(name="poolB", bufs=1) as poolB,
                tc.tile_pool(name="psB", bufs=1, space="PSUM") as psB,
            ):
                for chk in range(NCH):
                    c0 = chk * CH
                    # QA^T and ctx^T per head for this s-chunk
                    ctx_s = poolB.tile([128, NST, CH], f32r, tag="ctx", bufs=1)
                    for h in range(H):
                        hs = h * D
                        qat = poolB.tile([128, NCT, CH], f32r, tag="qat", bufs=1)
                        for et in range(NCT):
                            pqa = psB.tile([128, CH], f32, tag="pqa", bufs=1)
                            for kt in range(NCT):
                                nc.tensor.matmul(
                                    pqa[:],
                                    wqp_s[:, kt, hs + et * 128:hs + (et + 1) * 128],
                                    xT_s[:, kt, c0:c0 + CH],
                                    start=(kt == 0), stop=(kt == NCT - 1))
                            nc.scalar.activation(out=qat[:, et, :], in_=pqa[:],
                                                 func=AF.Copy)
                        for dt_ in range(NCT):
                            pct = psB.tile([128, CH], f32, tag="pct", bufs=1)
                            for et in range(NCT):
                                nc.tensor.matmul(
                                    pct[:], mut_s[:, et, h, dt_ * 128:(dt_ + 1) * 128],
                                    qat[:, et, :],
                                    start=(et == 0), stop=(et == NCT - 1))
                            nc.vector.tensor_scalar_mul(
                                out=ctx_s[:, h * NCT + dt_, :], in0=pct[:],
                                scalar1=recip_s[:, h, dt_:dt_ + 1])

                    # out-proj, transposed: woT [c, s-chunk]
                    wot = poolB.tile([128, NCT, CH], f32, tag="wft", bufs=1)
                    for ct in range(NCT):
                        pwo = psB.tile([128, CH], f32, tag="pwo", bufs=2)
                        for kt in range(NST):
                            nc.tensor.matmul(
                                pwo[:], wo_s[:, kt, ct * 128:(ct + 1) * 128],
                                ctx_s[:, kt, :],
                                start=(kt == 0), stop=(kt == NST - 1))
                        nc.scalar.activation(out=wot[:, ct, :], in_=pwo[:],
                                             func=AF.Copy)

                    # transpose back + residual + LN1 + LN2
                    out2_s = poolB.tile([128, NCH, D], f32, tag="out2", bufs=1)
                    for st in range(NCH):
                        gst = chk * NCH + st
                        t1 = poolB.tile([128, D], f32, tag="t1", bufs=2)
                        for ct in range(NCT):
                            ptr = psB.tile([128, 128], f32, tag="ptr", bufs=2)
                            nc.tensor.transpose(
                                ptr[:], wot[:, ct, st * 128:(st + 1) * 128], ident_s[:])
                            nc.vector.tensor_add(
                                out=t1[:, ct * 128:(ct + 1) * 128], in0=ptr[:],
                                in1=x_s[:, gst, ct * 128:(ct + 1) * 128])
                        o1 = poolB.tile([128, D], f32, tag="o1", bufs=2)
                        ln_block(poolB, t1[:], "g1b", "be1b", o1[:])
                        nc.vector.tensor_add(out=o1[:], in0=o1[:], in1=x_s[:, gst, :])
                        ln_block(poolB, o1[:], "g2b", "be2b", out2_s[:, st, :])

                    # transpose out2 chunk -> [c, s] for FFN1
                    o2t = poolB.tile([128, NCT, CH], f32r, tag="o2t", bufs=1)
                    for st in range(NCH):
                        for ct in range(NCT):
                            ptr = psB.tile([128, 128], f32, tag="ptr", bufs=2)
                            nc.tensor.transpose(
                                ptr[:], out2_s[:, st, ct * 128:(ct + 1) * 128], ident_s[:])
                            nc.vector.tensor_copy(
                                out=o2t[:, ct, st * 128:(st + 1) * 128], in_=ptr[:])

                    # FFN1 (+bias +relu), h1^T [j, s]
                    h1_s = poolB.tile([128, NST, CH], f32r, tag="h1", bufs=1)
                    for jt in range(NST):
                        ph1 = psB.tile([128, CH], f32, tag="ph1", bufs=1)
                        for kt in range(NCT):
                            nc.tensor.matmul(
                                ph1[:], w1_s[:, kt, jt * 128:(jt + 1) * 128],
                                o2t[:, kt, :],
                                start=(kt == 0), stop=(kt == NCT - 1))
                        nc.scalar.activation(out=h1_s[:, jt, :], in_=ph1[:],
                                             func=AF.Relu, bias=b1t_s[:, jt:jt + 1],
                                             scale=1.0)

                    # FFN2, transposed: f2T [c, s-chunk]
                    f2t = poolB.tile([128, NCT, CH], f32, tag="wft", bufs=1)
                    for ct in range(NCT):
                        pf2 = psB.tile([128, CH], f32, tag="pf2", bufs=1)
                        for jt in range(NST):
                            nc.tensor.matmul(
                                pf2[:], w2_s[:, jt, ct * 128:(ct + 1) * 128],
                                h1_s[:, jt, :],
                                start=(jt == 0), stop=(jt == NST - 1))
                        nc.scalar.activation(out=f2t[:, ct, :], in_=pf2[:],
                                             func=AF.Copy)

                    # transpose back + bias + residual + LN3 -> DMA
                    for st in range(NCH):
                        gst = chk * NCH + st
                        t2 = poolB.tile([128, D], f32, tag="t2", bufs=2)
                        for ct in range(NCT):
                            ptr = psB.tile([128, 128], f32, tag="ptr", bufs=2)
                            nc.tensor.transpose(
                                ptr[:], f2t[:, ct, st * 128:(st + 1) * 128], ident_s[:])
                            nc.vector.tensor_add(
                                out=t2[:, ct * 128:(ct + 1) * 128], in0=ptr[:],
                                in1=out2_s[:, st, ct * 128:(ct + 1) * 128])
                        nc.vector.tensor_add(out=t2[:], in0=t2[:], in1=bvec_s["b2b"][:])
                        oo = poolB.tile([128, D], f32, tag="oo", bufs=1)
                        ln_block(poolB, t2[:], "g3b", "be3b", oo[:])
                        nc.sync.dma_start(
                            out=out_d[gst * 128:(gst + 1) * 128, :], in_=oo[:])

    import concourse.mybir as mybir2
    _split_multi_waits(nc, mybir2)
    return nc


def _host_prep(inputs):
    x = np.asarray(inputs["enc_inputs"], dtype=np.float32)        # [B, S, D]
    A = np.asarray(inputs["A"], dtype=np.float32)                 # [D, D]
    Wk = np.asarray(inputs["Wk"], dtype=np.float32)               # [D, HID]
    Wq = np.asarray(inputs["Wq"], dtype=np.float32)
    scale = np.float32(1.0 / np.sqrt(D))
    WKP = np.einsum("chd,de->che", Wk.reshape(D, H, D), A).reshape(D, HID) * scale
    WQP = np.einsum("chd,de->che", Wq.reshape(D, H, D), A).reshape(D, HID)
    b1 = np.asarray(inputs["b1"], dtype=np.float32)
    b2 = np.asarray(inputs["b2"], dtype=np.float32)

    def bcast(v):
        return np.ascontiguousarray(
            np.broadcast_to(np.asarray(v, np.float32)[None, :], (128, D)))

    common = {
        "wkp": np.ascontiguousarray(WKP),
        "wqp": np.ascontiguousarray(WQP),
        "wv": np.ascontiguousarray(inputs["Wv"], dtype=np.float32),
        "wo": np.ascontiguousarray(inputs["Wo"], dtype=np.float32),
        "w1": np.ascontiguousarray(inputs["W1"], dtype=np.float32),
        "w2": np.ascontiguousarray(inputs["W2"], dtype=np.float32),
        "b1t": np.ascontiguousarray(b1.reshape(NST, 128).T),
        "b2b": bcast(b2),
        "g1b": bcast(inputs["ln_attn_g"]), "be1b": bcast(inputs["ln_attn_b"]),
        "g2b": bcast(inputs["ln_enc_g"]), "be2b": bcast(inputs["ln_enc_b"]),
        "g3b": bcast(inputs["ln_ffn_g"]), "be3b": bcast(inputs["ln_ffn_b"]),
    }
    in_maps = []
    for b in range(B):
        m = dict(common)
        m["x"] = np.ascontiguousarray(x[b])
        m["xT"] = np.ascontiguousarray(x[b].T)
        in_maps.append(m)
    return in_maps


def kernel(**inputs):
    import os
    from concourse.bass_utils import run_bass_kernel_spmd

    if "nc" not in _CACHE:
        _CACHE["nc"] = _build()
    nc = _CACHE["nc"]
    in_maps = _host_prep(inputs)
    trace = bool(os.environ.get("KERNEL_TRACE"))
    res = run_bass_kernel_spmd(nc, in_maps, list(range(B)), trace=trace,
                               tmpdir=os.environ.get("KERNEL_TRACE_DIR") or None)
    if trace:
        _CACHE["last_result"] = res
        if res.exec_time_ns is not None:
            print(f"HW exec time: {res.exec_time_ns} ns")
    out = np.stack([res.results[b]["out_o"] for b in range(B)])      # [B, S, D]
    attn = np.stack([res.results[b]["attn_o"] for b in range(B)])    # [B, H, D, S]
    return out, attn


# revision 13
# speedup vs baseline: 1.7763x; 1.7763x over previous
"""Trainium2 Bass kernel for nn_EncoderLayer_54116587929733.

Data-parallel over batch: B=8 batches -> 8 NeuronCores, no collectives.

Per-core math (batch b, S=2048, d=256, H=8 heads, hid=2048), per head h:
    scores = A^T K_h^T / sqrt(d)            [d, S]
    attn   = softmax_S(scores)              [d, S]   (output!)
    M      = attn @ V_h                     [d, d]
    QA     = Q_h @ A                        [S, d]
    ctx    = QA @ M^T                       [S, d]
then concat heads, out-proj + 3 LayerNorms + FFN.

Key restructurings (all exact algebra, no approximation beyond f32r):
  * A is folded into the K/Q projections on the host:
      WKP_h = Wk_h @ A / sqrt(d)  =>  scores = WKP_h^T x^T  (K never built)
      WQP_h = Wq_h @ A            =>  QA^T   = WQP_h^T x^T  (Q never built)
  * V is never materialized:  M^T = Wv_h^T (x^T ETu)  with
    G = x^T ETu computed first (S-contraction), then a tiny d x d matmul.
  * Softmax denominators ride a ones-vector matmul over ETu and are
    folded into the ctx evacuation as a per-partition row scale.
  * Heads are processed in pairs so every big matmul has moving dim 512.
  * Out-proj and FFN2 run transposed ([c, s] output, N=512) and are
    transposed back by PE-transpose blocks fused into the residual add.

All matmuls run in float32r: full fp32 operand width at 1 cycle/row on
the PE for moving dim >= 256 (~1.6e-4 rel err vs 2.6e-3 for bf16).
"""

import sys

sys.path.insert(0, "/opt/trn_rl_repo")

import numpy as np

S = 2048
D = 256
H = 8
B = 8
HID = 2048
EPS = 1e-5
NST = S // 128      # 16 s-tiles
NCT = D // 128      # 2 c/e/dl-tiles
NCH = 4             # s-chunks of 512
CH = S // NCH       # 512

_CACHE = {}


def _split_multi_waits(nc, mybir):
    """This walrus build supports only ONE sync-wait per instruction;
    Tile attaches several.  Move extras onto same-engine NOPs inserted
    just before the offending instruction."""
    cnt = 0
    for fn in nc.m.functions:
        for blk in fn.blocks:
            new_insts = []
            for inst in blk.instructions:
                si = inst.sync_info
                if si is not None and si.on_wait and len(si.on_wait) > 1:
                    waits = list(si.on_wait)
                    for w in waits[:-1]:
                        cnt += 1
                        new_insts.append(mybir.InstNoOp(
                            name=f"{inst.name}_wsplit{cnt}", ins=[], outs=[],
                            engine=inst.engine,
                            sync_info=mybir.SyncInfo(on_wait=[w], on_update=[])))
                    si.on_wait = waits[-1:]
                new_insts.append(inst)
            blk.instructions = new_insts
    return cnt


def _build():
    import concourse.bass as bass
    import concourse.mybir as mybir
    import concourse.tile as tile
    from concourse.masks import make_identity

    f32 = mybir.dt.float32
    f32r = mybir.dt.float32r
    AF = mybir.ActivationFunctionType

    nc = bass.Bass(target_bir_lowering=False)

    # ---- DRAM I/O --------------------------------------------------
    x_d = nc.dram_tensor("x", [S, D], f32r, kind="ExternalInput")
    xT_d = nc.dram_tensor("xT", [D, S], f32r, kind="ExternalInput")
    wkp_d = nc.dram_tensor("wkp", [D, HID], f32r, kind="ExternalInput")
    wqp_d = nc.dram_tensor("wqp", [D, HID], f32r, kind="ExternalInput")
    wv_d = nc.dram_tensor("wv", [D, HID], f32r, kind="ExternalInput")
    wo_d = nc.dram_tensor("wo", [HID, D], f32r, kind="ExternalInput")
    w1_d = nc.dram_tensor("w1", [D, HID], f32r, kind="ExternalInput")
    w2_d = nc.dram_tensor("w2", [HID, D], f32r, kind="ExternalInput")
    b1t_d = nc.dram_tensor("b1t", [128, NST], f32, kind="ExternalInput")
    bvec_names = ["b2b", "g1b", "be1b", "g2b", "be2b", "g3b", "be3b"]
    bvec_d = {n: nc.dram_tensor(n, [128, D], f32, kind="ExternalInput")
              for n in bvec_names}

    attn_d = nc.dram_tensor("attn_o", [H, D, S], f32, kind="ExternalOutput")
    out_d = nc.dram_tensor("out_o", [S, D], f32, kind="ExternalOutput")

    with tile.TileContext(nc) as tc:
        import contextlib
        ctx = contextlib.ExitStack()
        with ctx:
            persist = ctx.enter_context(tc.tile_pool(name="persist", bufs=1))

            xT_s = persist.tile([128, NCT, S], f32r)
            nc.sync.dma_start(out=xT_s[:], in_=xT_d.rearrange("(t p) s -> p t s", p=128))
            x_s = persist.tile([128, NST, D], f32r)
            nc.sync.dma_start(out=x_s[:], in_=x_d.rearrange("(t p) c -> p t c", p=128))
            wqp_s = persist.tile([128, NCT, HID], f32r)
            nc.sync.dma_start(out=wqp_s[:], in_=wqp_d.rearrange("(t p) n -> p t n", p=128))
            wo_s = persist.tile([128, NST, D], f32r)
            nc.sync.dma_start(out=wo_s[:], in_=wo_d.rearrange("(t p) c -> p t c", p=128))
            w1_s = persist.tile([128, NCT, HID], f32r)
            nc.sync.dma_start(out=w1_s[:], in_=w1_d.rearrange("(t p) n -> p t n", p=128))
            w2_s = persist.tile([128, NST, D], f32r)
            nc.sync.dma_start(out=w2_s[:], in_=w2_d.rearrange("(t p) c -> p t c", p=128))
            b1t_s = persist.tile([128, NST], f32)
            nc.sync.dma_start(out=b1t_s[:], in_=b1t_d[:])
            bvec_s = {}
            for n in bvec_names:
                bvec_s[n] = persist.tile([128, D], f32, name=f"bv_{n}", tag=f"bv_{n}")
                nc.sync.dma_start(out=bvec_s[n][:], in_=bvec_d[n][:])

            eps_s = persist.tile([128, 1], f32)
            nc.vector.memset(eps_s, EPS)
            ones_f = persist.tile([128, 1], f32)
            nc.vector.memset(ones_f, 1.0)
            ones_s = persist.tile([128, 1], f32r)
            nc.vector.tensor_copy(out=ones_s[:], in_=ones_f[:])
            ident_s = persist.tile([128, 128], f32)
            make_identity(nc, ident_s)

            # A->B carriers
            mut_s = persist.tile([128, NCT, H, D], f32r)     # [j, jt, h, dl]
            recip_s = persist.tile([128, H, NCT], f32)       # [dl, h, dlt]

            # ------------- Phase A: K/V side, head PAIRS -------------
            with (
                tc.tile_pool(name="poolA", bufs=1) as poolA,
                tc.tile_pool(name="psA", bufs=1, space="PSUM") as psA,
            ):
                wkp_s = poolA.tile([128, NCT, HID], f32r, tag="wkp")
                nc.sync.dma_start(out=wkp_s[:], in_=wkp_d.rearrange("(t p) n -> p t n", p=128))
                wv_s = poolA.tile([128, NCT, HID], f32r, tag="wv")
                nc.sync.dma_start(out=wv_s[:], in_=wv_d.rearrange("(t p) n -> p t n", p=128))

                for pr in range(H // 2):
                    h0 = 2 * pr
                    ps0 = h0 * D                 # pair's column offset in wkp
                    # scores^T for the PAIR: ETu [s, 512] = exp(x WKP_pair)
                    etu_s = poolA.tile([128, NST, 2 * D], f32r, tag="etu", bufs=1)
                    for st in range(NST):
                        stp = psA.tile([128, 2 * D], f32, tag="stp", bufs=2)
                        for kt in range(NCT):
                            nc.tensor.matmul(
                                stp[:], xT_s[:, kt, st * 128:(st + 1) * 128],
                                wkp_s[:, kt, ps0:ps0 + 2 * D],
                                start=(kt == 0), stop=(kt == NCT - 1))
                        nc.scalar.activation(out=etu_s[:, st, :], in_=stp[:],
                                             func=AF.Exp)

                    # denominators for the pair: [1, 512]
                    pd = psA.tile([1, 2 * D], f32, tag="pdx", bufs=1)
                    for st in range(NST):
                        nc.tensor.matmul(pd[:], ones_s[:, :1], etu_s[:, st, :],
                                         start=(st == 0), stop=(st == NST - 1))
                    denr = poolA.tile([1, 2 * D], f32, tag="denr", bufs=1)
                    nc.vector.tensor_copy(out=denr[:], in_=pd[:])
                    for q in range(4):           # q = hh*2 + et
                        hh, et = divmod(q, 2)
                        pdt = psA.tile([128, 1], f32, tag="pdx", bufs=1)
                        nc.tensor.matmul(pdt[:], denr[:1, q * 128:(q + 1) * 128],
                                         ones_f[:1, :1], start=True, stop=True)
                        nc.vector.reciprocal(out=recip_s[:, h0 + hh, et:et + 1],
                                             in_=pdt[:])

                    # G = x^T @ ETu  [c, 512]  (the V-side contraction over S)
                    g_s = poolA.tile([128, NCT, 2 * D], f32r, tag="g2", bufs=1)
                    for ct in range(NCT):
                        pg = psA.tile([128, 2 * D], f32, tag="pg", bufs=2)
                        for st in range(NST):
                            nc.tensor.matmul(
                                pg[:], x_s[:, st, ct * 128:(ct + 1) * 128],
                                etu_s[:, st, :],
                                start=(st == 0), stop=(st == NST - 1))
                        nc.vector.tensor_copy(out=g_s[:, ct, :], in_=pg[:])

                    # MuT_h = Wv_h^T @ G_h   [j, dl]  (tiny d x d matmuls)
                    for hh in range(2):
                        h = h0 + hh
                        for jt in range(NCT):
                            pmu = psA.tile([128, D], f32, tag="pmu", bufs=1)
                            for ct in range(NCT):
                                nc.tensor.matmul(
                                    pmu[:],
                                    wv_s[:, ct, h * D + jt * 128:h * D + (jt + 1) * 128],
                                    g_s[:, ct, hh * D:(hh + 1) * D],
                                    start=(ct == 0), stop=(ct == NCT - 1))
                            nc.vector.tensor_copy(out=mut_s[:, jt, h, :], in_=pmu[:])

                    # scores [e, s] per head -> E -> normalize -> attn out
                    for hh in range(2):
                        h = h0 + hh
                        hs = h * D
                        e_s = poolA.tile([128, NCT, S], f32, tag="e", bufs=1)
                        for et in range(NCT):
                            for chk in range(NCH):
                                sc = psA.tile([128, CH], f32, tag="sc", bufs=2)
                                for kt in range(NCT):
                                    nc.tensor.matmul(
                                        sc[:],
                                        wkp_s[:, kt, hs + et * 128:hs + (et + 1) * 128],
                                        xT_s[:, kt, chk * CH:(chk + 1) * CH],
                                        start=(kt == 0), stop=(kt == NCT - 1))
                                nc.scalar.activation(
                                    out=e_s[:, et, chk * CH:(chk + 1) * CH],
                                    in_=sc[:], func=AF.Exp)
                        for et in range(NCT):
                            nc.vector.tensor_scalar_mul(
                                out=e_s[:, et, :], in0=e_s[:, et, :],
                                scalar1=recip_s[:, h, et:et + 1])
                            nc.sync.dma_start(
                                out=attn_d[h, et * 128:(et + 1) * 128, :],
                                in_=e_s[:, et, :])

            # ---------- Phase B: Q side + out-proj + FFN, per s-chunk ----------
            def ln_block(pool, src_f32_ap, gname, bname, dst_ap):
                """dst = LN(src) * g + b   (src: [128, D] f32 SBUF)"""
                stats = pool.tile([128, 6], f32, tag="lnstats", bufs=4)
                nc.vector.bn_stats(out=stats[:], in_=src_f32_ap)
                mv = pool.tile([128, 2], f32, tag="lnmv", bufs=4)
                nc.vector.bn_aggr(out=mv[:], in_=stats[:])
                nc.scalar.activation(out=mv[:, 1:2], in_=mv[:, 1:2],
                                     func=AF.Sqrt, bias=eps_s[:], scale=1.0)
                nc.vector.reciprocal(out=mv[:, 1:2], in_=mv[:, 1:2])
                nc.vector.tensor_scalar(
                    out=dst_ap, in0=src_f32_ap,
                    scalar1=mv[:, 0:1], scalar2=mv[:, 1:2],
                    op0=mybir.AluOpType.subtract, op1=mybir.AluOpType.mult)
                nc.vector.tensor_mul(out=dst_ap, in0=dst_ap, in1=bvec_s[gname][:])
                nc.vector.tensor_add(out=dst_ap, in0=dst_ap, in1=bvec_s[bname][:])

            with (
                tc.tile_pool(name="poolB", bufs=1) as poolB,
                tc.tile_pool(name="psB", bufs=1, space="PSUM") as psB,
            ):
                for chk in range(NCH):
                    c0 = chk * CH
                    # QA^T and ctx^T per head for this s-chunk
                    ctx_s = poolB.tile([128, NST, CH], f32r, tag="ctx", bufs=1)
                    for h in range(H):
                        hs = h * D
                        qat = poolB.tile([128, NCT, CH], f32r, tag="qat", bufs=1)
                        for et in range(NCT):
                            pqa = psB.tile([128, CH], f32, tag="pqa", bufs=1)
                            for kt in range(NCT):
                                nc.tensor.matmul(
                                    pqa[:],
                                    wqp_s[:, kt, hs + et * 128:hs + (et + 1) * 128],
                                    xT_s[:, kt, c0:c0 + CH],
                                    start=(kt == 0), stop=(kt == NCT - 1))
                            nc.scalar.activation(out=qat[:, et, :], in_=pqa[:],
                                                 func=AF.Copy)
                        for dt_ in range(NCT):
                            pct = psB.tile([128, CH], f32, tag="pct", bufs=1)
                            for et in range(NCT):
                                nc.tensor.matmul(
                                    pct[:], mut_s[:, et, h, dt_ * 128:(dt_ + 1) * 128],
                                    qat[:, et, :],
                                    start=(et == 0), stop=(et == NCT - 1))
                            nc.vector.tensor_scalar_mul(
                                out=ctx_s[:, h * NCT + dt_, :], in0=pct[:],
                                scalar1=recip_s[:, h, dt_:dt_ + 1])

                    # out-proj, transposed: woT [c, s-chunk]
                    wot = poolB.tile([128, NCT, CH], f32, tag="wft", bufs=1)
                    for ct in range(NCT):
                        pwo = psB.tile([128, CH], f32, tag="pwo", bufs=2)
                        for kt in range(NST):
                            nc.tensor.matmul(
                                pwo[:], wo_s[:, kt, ct * 128:(ct + 1) * 128],
                                ctx_s[:, kt, :],
                                start=(kt == 0), stop=(kt == NST - 1))
                        nc.scalar.activation(out=wot[:, ct, :], in_=pwo[:],
                                             func=AF.Copy)

                    # transpose back + residual + LN1 + LN2
                    out2_s = poolB.tile([128, NCH, D], f32, tag="out2", bufs=1)
                    for st in range(NCH):
                        gst = chk * NCH + st
                        t1 = poolB.tile([128, D], f32, tag="t1", bufs=2)
                        for ct in range(NCT):
                            ptr = psB.tile([128, 128], f32, tag="ptr", bufs=2)
                            nc.tensor.transpose(
                                ptr[:], wot[:, ct, st * 128:(st + 1) * 128], ident_s[:])
                            nc.vector.tensor_add(
                                out=t1[:, ct * 128:(ct + 1) * 128], in0=ptr[:],
                                in1=x_s[:, gst, ct * 128:(ct + 1) * 128])
                        o1 = poolB.tile([128, D], f32, tag="o1", bufs=2)
                        ln_block(poolB, t1[:], "g1b", "be1b", o1[:])
                        nc.vector.tensor_add(out=o1[:], in0=o1[:], in1=x_s[:, gst, :])
                        ln_block(poolB, o1[:], "g2b", "be2b", out2_s[:, st, :])

                    # transpose out2 chunk -> [c, s] for FFN1
                    o2t = poolB.tile([128, NCT, CH], f32r, tag="o2t", bufs=1)
                    for st in range(NCH):
                        for ct in range(NCT):
                            ptr = psB.tile([128, 128], f32, tag="ptr", bufs=2)
                            nc.tensor.transpose(
                                ptr[:], out2_s[:, st, ct * 128:(ct + 1) * 128], ident_s[:])
                            nc.vector.tensor_copy(
                                out=o2t[:, ct, st * 128:(st + 1) * 128], in_=ptr[:])

                    # FFN1 (+bias +relu), h1^T [j, s]
                    h1_s = poolB.tile([128, NST, CH], f32r, tag="h1", bufs=1)
                    for jt in range(NST):
                        ph1 = psB.tile([128, CH], f32, tag="ph1", bufs=1)
                        for kt in range(NCT):
                            nc.tensor.matmul(
                                ph1[:], w1_s[:, kt, jt * 128:(jt + 1) * 128],
                                o2t[:, kt, :],
                                start=(kt == 0), stop=(kt == NCT - 1))
                        nc.scalar.activation(out=h1_s[:, jt, :], in_=ph1[:],
                                             func=AF.Relu, bias=b1t_s[:, jt:jt + 1],
                                             scale=1.0)

                    # FFN2, transposed: f2T [c, s-chunk]
                    f2t = poolB.tile([128, NCT, CH], f32, tag="wft", bufs=1)
                    for ct in range(NCT):
                        pf2 = psB.tile([128, CH], f32, tag="pf2", bufs=1)
                        for jt in range(NST):
                            nc.tensor.matmul(
                                pf2[:], w2_s[:, jt, ct * 128:(ct + 1) * 128],
                                h1_s[:, jt, :],
                                start=(jt == 0), stop=(jt == NST - 1))
                        nc.scalar.activation(out=f2t[:, ct, :], in_=pf2[:],
                                             func=AF.Copy)

                    # transpose back + bias + residual + LN3 -> DMA
                    for st in range(NCH):
                        gst = chk * NCH + st
                        t2 = poolB.tile([128, D], f32, tag="t2", bufs=2)
                        for ct in range(NCT):
                            ptr = psB.tile([128, 128], f32, tag="ptr", bufs=2)
                            nc.tensor.transpose(
                                ptr[:], f2t[:, ct, st * 128:(st + 1) * 128], ident_s[:])
                            nc.vector.tensor_add(
                                out=t2[:, ct * 128:(ct + 1) * 128], in0=ptr[:],
                                in1=out2_s[:, st, ct * 128:(ct + 1) * 128])
                        nc.vector.tensor_add(out=t2[:], in0=t2[:], in1=bvec_s["b2b"][:])
                        oo = poolB.tile([128, D], f32, tag="oo", bufs=1)
                        ln_block(poolB, t2[:], "g3b", "be3b", oo[:])
                        nc.sync.dma_start(
                            out=out_d[gst * 128:(gst + 1) * 128, :], in_=oo[:])

    import concourse.mybir as mybir2
    _split_multi_waits(nc, mybir2)
    return nc


def _host_prep(inputs):
    x = np.asarray(inputs["enc_inputs"], dtype=np.float32)        # [B, S, D]
    A = np.asarray(inputs["A"], dtype=np.float32)                 # [D, D]
    Wk = np.asarray(inputs["Wk"], dtype=np.float32)               # [D, HID]
    Wq = np.asarray(inputs["Wq"], dtype=np.float32)
    scale = np.float32(1.0 / np.sqrt(D))
    WKP = np.einsum("chd,de->che", Wk.reshape(D, H, D), A).reshape(D, HID) * scale
    WQP = np.einsum("chd,de->che", Wq.reshape(D, H, D), A).reshape(D, HID)
    b1 = np.asarray(inputs["b1"], dtype=np.float32)
    b2 = np.asarray(inputs["b2"], dtype=np.float32)

    def bcast(v):
        return np.ascontiguousarray(
            np.broadcast_to(np.asarray(v, np.float32)[None, :], (128, D)))

    common = {
        "wkp": np.ascontiguousarray(WKP),
        "wqp": np.ascontiguousarray(WQP),
        "wv": np.ascontiguousarray(inputs["Wv"], dtype=np.float32),
        "wo": np.ascontiguousarray(inputs["Wo"], dtype=np.float32),
        "w1": np.ascontiguousarray(inputs["W1"], dtype=np.float32),
        "w2": np.ascontiguousarray(inputs["W2"], dtype=np.float32),
        "b1t": np.ascontiguousarray(b1.reshape(NST, 128).T),
        "b2b": bcast(b2),
        "g1b": bcast(inputs["ln_attn_g"]), "be1b": bcast(inputs["ln_attn_b"]),
        "g2b": bcast(inputs["ln_enc_g"]), "be2b": bcast(inputs["ln_enc_b"]),
        "g3b": bcast(inputs["ln_ffn_g"]), "be3b": bcast(inputs["ln_ffn_b"]),
    }
    in_maps = []
    for b in range(B):
        m = dict(common)
        m["x"] = np.ascontiguousarray(x[b])
        m["xT"] = np.ascontiguousarray(x[b].T)
        in_maps.append(m)
    return in_maps


def kernel(**inputs):
    import os
    from concourse.bass_utils import run_bass_kernel_spmd

    if "nc" not in _CACHE:
        _CACHE["nc"] = _build()
    nc = _CACHE["nc"]
    in_maps = _host_prep(inputs)
    trace = bool(os.environ.get("KERNEL_TRACE"))
    res = run_bass_kernel_spmd(nc, in_maps, list(range(B)), trace=trace,
                               tmpdir=os.environ.get("KERNEL_TRACE_DIR") or None)
    if trace:
        _CACHE["last_result"] = res
        if res.exec_time_ns is not None:
            print(f"HW exec time: {res.exec_time_ns} ns")
    out = np.stack([res.results[b]["out_o"] for b in range(B)])      # [B, S, D]
    attn = np.stack([res.results[b]["attn_o"] for b in range(B)])    # [B, H, D, S]
    return out, attn


# revision 29
# speedup vs baseline: 3.2400x; 1.8240x over previous
"""Trainium2 Bass kernel for nn_EncoderLayer_54116587929733.

Data-parallel over batch: B=8 batches -> 8 NeuronCores, no collectives.

Per-core math (batch b, S=2048, d=256, H=8 heads, hid=2048), per head h:
    scores = A^T K_h^T / sqrt(d)            [d, S]
    attn   = softmax_S(scores)              [d, S]   (output!)
    M      = attn @ V_h                     [d, d]
    QA     = Q_h @ A                        [S, d]
    ctx    = QA @ M^T                       [S, d]
then concat heads, out-proj + 3 LayerNorms + FFN.

Key restructurings (exact algebra):
  * A folded into the K/Q projections on the host:
      WKP_h = Wk_h @ A / sqrt(d)  =>  scores = WKP_h^T x^T  (K never built)
      WQP_h = Wq_h @ A            =>  QA^T   = WQP_h^T x^T  (Q never built)
  * V never materialized:  M^T = Wv_h^T (x^T ETu), with G = x^T ETu
    (S-contraction) first, then a tiny d x d matmul.
  * Softmax denominators ride a ones-vector matmul over ETu; the
    normalization is folded into the scores-exp as a per-partition bias
    exp(s + ln(1/denom)) and into a per-partition scale of WMU (below).
  * The whole Q side is LINEAR in x given the (runtime) attention
    matrices, so it collapses to ONE 256x256 matrix:
      out_attn = x @ W*,   W* = sum_h WQP_h MuT_h diag(1/denom) Wo_h
    computed on-chip from tiny d x d matmuls.
  * Heads processed in pairs so the scores^T/G matmuls run at N=512.
  * Out-proj and FFN2 run transposed ([c, s] out, N=512) and transpose
    back via PE-transpose blocks fused into the residual adds.

Everything runs in fp16 (fp32 PSUM accumulation): ~1e-3 accuracy at
full PE speed with fast-weight-load.  All value magnitudes were checked
to sit inside fp16 normal range.
"""

import sys

sys.path.insert(0, "/opt/trn_rl_repo")

import numpy as np

S = 2048
D = 256
H = 8
B = 8
HID = 2048
EPS = 1e-5
NST = S // 128      # 16 s-tiles
NCT = D // 128      # 2 c/e/dl-tiles
NCH = 4             # s-chunks of 512
CH = S // NCH       # 512

_CACHE = {}


def _split_multi_waits(nc, mybir):
    """This walrus build supports only ONE sync-wait per instruction;
    Tile attaches several.  Move extras onto same-engine NOPs inserted
    just before the offending instruction."""
    cnt = 0
    for fn in nc.m.functions:
        for blk in fn.blocks:
            new_insts = []
            for inst in blk.instructions:
                si = inst.sync_info
                if si is not None and si.on_wait and len(si.on_wait) > 1:
                    waits = list(si.on_wait)
                    for w in waits[:-1]:
                        cnt += 1
                        new_insts.append(mybir.InstNoOp(
                            name=f"{inst.name}_wsplit{cnt}", ins=[], outs=[],
                            engine=inst.engine,
                            sync_info=mybir.SyncInfo(on_wait=[w], on_update=[])))
                    si.on_wait = waits[-1:]
                new_insts.append(inst)
            blk.instructions = new_insts
    return cnt


def _build():
    import concourse.bass as bass
    import concourse.mybir as mybir
    import concourse.tile as tile
    from concourse.masks import make_identity

    f32 = mybir.dt.float32
    f16 = mybir.dt.float16
    AF = mybir.ActivationFunctionType

    nc = bass.Bass(target_bir_lowering=False)

    # ---- DRAM I/O (fp16 operands, fp32 outputs) --------------------
    xh_d = nc.dram_tensor("xh", [S, D], f16, kind="ExternalInput")
    xTh_d = nc.dram_tensor("xTh", [D, S], f16, kind="ExternalInput")
    wkp_d = nc.dram_tensor("wkp", [D, HID], f16, kind="ExternalInput")
    wv_d = nc.dram_tensor("wv", [D, HID], f16, kind="ExternalInput")
    wqpt_d = nc.dram_tensor("wqpt", [HID, D], f16, kind="ExternalInput")
    wo_d = nc.dram_tensor("wo", [HID, D], f16, kind="ExternalInput")
    w1_d = nc.dram_tensor("w1", [D, HID], f16, kind="ExternalInput")
    w2_d = nc.dram_tensor("w2", [HID, D], f16, kind="ExternalInput")
    b1t_d = nc.dram_tensor("b1t", [128, NST], f32, kind="ExternalInput")
    bvec_names = ["b2b", "g1b", "be1b", "g2b", "be2b", "g3b", "be3b"]
    bvec_d = {n: nc.dram_tensor(n, [128, D], f16, kind="ExternalInput")
              for n in bvec_names}

    attn_d = nc.dram_tensor("attn_o", [H, D, S], f32, kind="ExternalOutput")
    out_d = nc.dram_tensor("out_o", [S, D], f32, kind="ExternalOutput")

    with tile.TileContext(nc) as tc:
        import contextlib
        ctx = contextlib.ExitStack()
        with ctx:
            persist = ctx.enter_context(tc.tile_pool(name="persist", bufs=1))

            # attention-phase operands first (compute starts on these)
            xTh_s = persist.tile([128, NCT, S], f16)
            nc.sync.dma_start(out=xTh_s[:], in_=xTh_d.rearrange("(t p) s -> p t s", p=128))
            xh_s = persist.tile([128, NST, D], f16)
            nc.sync.dma_start(out=xh_s[:], in_=xh_d.rearrange("(t p) c -> p t c", p=128))
            wqpt_s = persist.tile([128, NST, D], f16)
            wo_s = persist.tile([128, NST, D], f16)
            w1_s = persist.tile([128, NCT, HID], f16)
            nc.scalar.dma_start(out=w1_s[:], in_=w1_d.rearrange("(t p) n -> p t n", p=128))
            w2_s = persist.tile([128, NST, D], f16)
            nc.scalar.dma_start(out=w2_s[:], in_=w2_d.rearrange("(t p) c -> p t c", p=128))
            b1t_s = persist.tile([128, NST], f32)
            nc.scalar.dma_start(out=b1t_s[:], in_=b1t_d[:])
            bvec_s = {}
            for n in bvec_names:
                bvec_s[n] = persist.tile([128, D], f16, name=f"bv_{n}", tag=f"bv_{n}")
                nc.scalar.dma_start(out=bvec_s[n][:], in_=bvec_d[n][:])

            eps_s = persist.tile([128, 1], f32)
            nc.vector.memset(eps_s, EPS)
            ones_f = persist.tile([128, 1], f32)
            nc.vector.memset(ones_f, 1.0)
            zero_s = persist.tile([128, 1], f32)
            nc.vector.memset(zero_s, 0.0)
            ones_h = persist.tile([128, 1], f16)
            nc.vector.tensor_copy(out=ones_h[:], in_=ones_f[:])
            ident_h = persist.tile([128, 128], f16)
            make_identity(nc, ident_h)

            # A->B carriers
            mut_s = persist.tile([128, NCT, H, D], f16)      # [j, jt, h, dl]
            recip_s = persist.tile([128, H, NCT], f32)       # 1/denom [dl, h, dlt]
            wmut_s = persist.tile([128, NCT, H, D], f16)     # (WQP_h MuT_h)^T * recip
            wst_s = persist.tile([128, NCT, D], f16)         # W*

            # ------------- Phase A: K/V side, head PAIRS -------------
            with (
                tc.tile_pool(name="poolA", bufs=1) as poolA,
                tc.tile_pool(name="psA", bufs=1, space="PSUM") as psA,
            ):
                wkp_s = poolA.tile([128, NCT, HID], f16, tag="wkp")
                wkp_r = wkp_d.rearrange("(t p) n -> p t n", p=128)
                for _pr in range(H // 2):
                    nc.sync.dma_start(
                        out=wkp_s[:, :, _pr * 2 * D:(_pr + 1) * 2 * D],
                        in_=wkp_r[:, :, _pr * 2 * D:(_pr + 1) * 2 * D])
                wv_s = poolA.tile([128, NCT, HID], f16, tag="wv")
                nc.sync.dma_start(out=wv_s[:], in_=wv_d.rearrange("(t p) n -> p t n", p=128))
                nc.scalar.dma_start(out=wqpt_s[:], in_=wqpt_d.rearrange("(t p) c -> p t c", p=128))
                nc.scalar.dma_start(out=wo_s[:], in_=wo_d.rearrange("(t p) c -> p t c", p=128))

                for pr in range(H // 2):
                    h0 = 2 * pr
                    ps0 = h0 * D
                    # scores^T for the PAIR: ETu [s, 512] = exp(x WKP_pair)
                    etu_s = poolA.tile([128, NST, 2 * D], f16, tag="etu", bufs=2)
                    for st in range(NST):
                        stp = psA.tile([128, 2 * D], f32, tag="stp", bufs=2)
                        for kt in range(NCT):
                            nc.tensor.matmul(
                                stp[:], xTh_s[:, kt, st * 128:(st + 1) * 128],
                                wkp_s[:, kt, ps0:ps0 + 2 * D],
                                start=(kt == 0), stop=(kt == NCT - 1))
                        nc.scalar.activation(out=etu_s[:, st, :], in_=stp[:],
                                             func=AF.Exp)

                    # pair denominators: [1, 512] = sum_s ETu
                    pd = psA.tile([1, 2 * D], f32, tag="pdx", bufs=1)
                    for st in range(NST):
                        nc.tensor.matmul(pd[:], ones_h[:, :1], etu_s[:, st, :],
                                         start=(st == 0), stop=(st == NST - 1))
                    denr = poolA.tile([1, 2 * D], f32, tag="denr", bufs=1)
                    nc.vector.tensor_copy(out=denr[:], in_=pd[:])
                    for q in range(4):           # q = hh*2 + et
                        hh, et = divmod(q, 2)
                        pdt = psA.tile([128, 1], f32, tag="pdx", bufs=1)
                        nc.tensor.matmul(pdt[:], denr[:1, q * 128:(q + 1) * 128],
                                         ones_f[:1, :1], start=True, stop=True)
                        nc.vector.reciprocal(out=recip_s[:, h0 + hh, et:et + 1],
                                             in_=pdt[:])

                    # G = x^T @ ETu  [c, 512]  (V-side contraction over S)
                    g_s = poolA.tile([128, NCT, 2 * D], f16, tag="g2", bufs=2)
                    for ct in range(NCT):
                        pg = psA.tile([128, 2 * D], f32, tag="pg", bufs=2)
                        for st in range(NST):
                            nc.tensor.matmul(
                                pg[:], xh_s[:, st, ct * 128:(ct + 1) * 128],
                                etu_s[:, st, :],
                                start=(st == 0), stop=(st == NST - 1))
                        nc.vector.tensor_copy(out=g_s[:, ct, :], in_=pg[:])

                    # MuT_h = Wv_h^T @ G_h   [j, dl]
                    for hh in range(2):
                        h = h0 + hh
                        for jt in range(NCT):
                            pmu = psA.tile([128, D], f32, tag="pmu", bufs=1)
                            for ct in range(NCT):
                                nc.tensor.matmul(
                                    pmu[:],
                                    wv_s[:, ct, h * D + jt * 128:h * D + (jt + 1) * 128],
                                    g_s[:, ct, hh * D:(hh + 1) * D],
                                    start=(ct == 0), stop=(ct == NCT - 1))
                            nc.vector.tensor_copy(out=mut_s[:, jt, h, :], in_=pmu[:])
                    # attention output: E = transpose(ETu) * recip
                    for hh in range(2):
                        h = h0 + hh
                        e_s = poolA.tile([128, NCT, S], f16, tag="e", bufs=2)
                        for et in range(NCT):
                            ecol = hh * D + et * 128
                            for sb in range(2):
                                pte = psA.tile([128, 8 * 128], f16, tag="pte",
                                               bufs=2)
                                for st8 in range(8):
                                    st = sb * 8 + st8
                                    nc.tensor.transpose(
                                        pte[:, st8 * 128:(st8 + 1) * 128],
                                        etu_s[:, st, ecol:ecol + 128],
                                        ident_h[:])
                                nc.vector.tensor_scalar_mul(
                                    out=e_s[:, et, sb * 1024:(sb + 1) * 1024],
                                    in0=pte[:],
                                    scalar1=recip_s[:, h, et:et + 1])
                            nc.gpsimd.dma_start(
                                out=attn_d[h, et * 128:(et + 1) * 128, :],
                                in_=e_s[:, et, :])


            # Collapse the whole Q side:  out_attn = x @ W*  with
            #   W* = sum_h WQP_h MuT_h diag(recip) Wo_h   (256 x 256, runtime)
            # recip is folded into the WMUT evacuation (per-partition scale).
            with tc.tile_pool(name="psW", bufs=1, space="PSUM") as psW:
                for h in range(H):
                    for dlt in range(NCT):
                        pwm = psW.tile([128, D], f32, tag="pwm", bufs=2)
                        for et in range(NCT):
                            nc.tensor.matmul(
                                pwm[:],
                                mut_s[:, et, h, dlt * 128:(dlt + 1) * 128],
                                wqpt_s[:, h * NCT + et, :],
                                start=(et == 0), stop=(et == NCT - 1))
                        nc.vector.tensor_scalar_mul(
                            out=wmut_s[:, dlt, h, :], in0=pwm[:],
                            scalar1=recip_s[:, h, dlt:dlt + 1])
                for ct in range(NCT):
                    pws = psW.tile([128, D], f32, tag="pws", bufs=2)
                    nkt = H * NCT
                    for h in range(H):
                        for dlt in range(NCT):
                            ki = h * NCT + dlt
                            nc.tensor.matmul(
                                pws[:],
                                wmut_s[:, dlt, h, ct * 128:(ct + 1) * 128],
                                wo_s[:, h * NCT + dlt, :],
                                start=(ki == 0), stop=(ki == nkt - 1))
                    nc.vector.tensor_copy(out=wst_s[:, ct, :], in_=pws[:])

            # ---------- Phase B: out-proj + FFN, per s-chunk ----------
            def ln_block(pool, src_ap, gname, bname, dst_ap):
                """dst = LN(src) * g + b   (fp16 chain, stats fp32)"""
                stats = pool.tile([128, 6], f32, tag="lnstats", bufs=8)
                nc.vector.bn_stats(out=stats[:], in_=src_ap)
                mv = pool.tile([128, 2], f32, tag="lnmv", bufs=8)
                nc.vector.bn_aggr(out=mv[:], in_=stats[:])
                nc.scalar.activation(out=mv[:, 1:2], in_=mv[:, 1:2],
                                     func=AF.Sqrt, bias=eps_s[:], scale=1.0)
                nc.vector.reciprocal(out=mv[:, 1:2], in_=mv[:, 1:2])
                y = pool.tile([128, D], f16, tag="lny", bufs=6)
                nc.vector.tensor_scalar(
                    out=y[:], in0=src_ap,
                    scalar1=mv[:, 0:1], scalar2=mv[:, 1:2],
                    op0=mybir.AluOpType.subtract, op1=mybir.AluOpType.mult)
                nc.vector.tensor_mul(out=y[:], in0=y[:], in1=bvec_s[gname][:])
                nc.vector.tensor_add(out=dst_ap, in0=y[:], in1=bvec_s[bname][:])

            with (
                tc.tile_pool(name="poolB", bufs=1) as poolB,
                tc.tile_pool(name="psB", bufs=1, space="PSUM") as psB,
            ):
                # B1: out-proj for the whole S:  woT [c', S] = W*^T x^T
                wot = poolB.tile([128, NCT, S], f16, tag="wot", bufs=1)
                for chk in range(NCH):
                    c0 = chk * CH
                    for ct in range(NCT):
                        pwo = psB.tile([128, CH], f32, tag="pwo", bufs=2)
                        for kt in range(NCT):
                            nc.tensor.matmul(
                                pwo[:], wst_s[:, kt, ct * 128:(ct + 1) * 128],
                                xTh_s[:, kt, c0:c0 + CH],
                                start=(kt == 0), stop=(kt == NCT - 1))
                        nc.scalar.activation(out=wot[:, ct, c0:c0 + CH],
                                             in_=pwo[:], func=AF.Copy)

                # B2: transpose back + residual + LN1 + LN2 (per s-tile)
                out2_s = poolB.tile([128, NST, D], f16, tag="out2", bufs=1)
                for st in range(NST):
                    t1 = poolB.tile([128, D], f16, tag="t1", bufs=4)
                    ptr = psB.tile([128, D], f16, tag="ptr", bufs=2)
                    for ct in range(NCT):
                        nc.tensor.transpose(
                            ptr[:, ct * 128:(ct + 1) * 128],
                            wot[:, ct, st * 128:(st + 1) * 128], ident_h[:])
                    nc.vector.tensor_add(out=t1[:], in0=ptr[:],
                                         in1=xh_s[:, st, :])
                    o1 = poolB.tile([128, D], f16, tag="o1", bufs=4)
                    ln_block(poolB, t1[:], "g1b", "be1b", o1[:])
                    t1b = poolB.tile([128, D], f16, tag="t1b", bufs=4)
                    nc.vector.tensor_add(out=t1b[:], in0=o1[:],
                                         in1=xh_s[:, st, :])
                    ln_block(poolB, t1b[:], "g2b", "be2b", out2_s[:, st, :])

                # B3: transpose out2 -> [c, S] for FFN1
                o2t = poolB.tile([128, NCT, S], f16, tag="o2t", bufs=1)
                for st in range(NST):
                    ptr = psB.tile([128, D], f16, tag="ptr", bufs=2)
                    for ct in range(NCT):
                        nc.tensor.transpose(
                            ptr[:, ct * 128:(ct + 1) * 128],
                            out2_s[:, st, ct * 128:(ct + 1) * 128], ident_h[:])
                    for ct in range(NCT):
                        nc.scalar.activation(
                            out=o2t[:, ct, st * 128:(st + 1) * 128],
                            in_=ptr[:, ct * 128:(ct + 1) * 128], func=AF.Copy)

                # B4/B5: FFN per s-chunk (FFN2 of chunk k overlaps FFN1 of k+1)
                f2t = poolB.tile([128, NCT, S], f16, tag="f2t", bufs=1)
                for chk in range(NCH):
                    c0 = chk * CH
                    h1c = poolB.tile([128, NST, CH], f16, tag="h1", bufs=2)
                    for jt in range(NST):
                        ph1 = psB.tile([128, CH], f32, tag="ph1", bufs=2)
                        for kt in range(NCT):
                            nc.tensor.matmul(
                                ph1[:], w1_s[:, kt, jt * 128:(jt + 1) * 128],
                                o2t[:, kt, c0:c0 + CH],
                                start=(kt == 0), stop=(kt == NCT - 1))
                        if jt % 2 == 0:
                            nc.scalar.activation(out=h1c[:, jt, :], in_=ph1[:],
                                                 func=AF.Relu,
                                                 bias=b1t_s[:, jt:jt + 1],
                                                 scale=1.0)
                        else:
                            nc.vector.tensor_scalar(
                                out=h1c[:, jt, :], in0=ph1[:],
                                scalar1=b1t_s[:, jt:jt + 1], scalar2=zero_s[:],
                                op0=mybir.AluOpType.add,
                                op1=mybir.AluOpType.max)
                    for ct in range(NCT):
                        pf2 = psB.tile([128, CH], f32, tag="pf2", bufs=2)
                        for jt in range(NST):
                            nc.tensor.matmul(
                                pf2[:], w2_s[:, jt, ct * 128:(ct + 1) * 128],
                                h1c[:, jt, :],
                                start=(jt == 0), stop=(jt == NST - 1))
                        nc.scalar.activation(out=f2t[:, ct, c0:c0 + CH],
                                             in_=pf2[:], func=AF.Copy)

                # B6: transpose back + bias + residual + LN3 -> DMA
                for st in range(NST):
                    t2 = poolB.tile([128, D], f16, tag="t2", bufs=4)
                    ptr = psB.tile([128, D], f16, tag="ptr", bufs=2)
                    for ct in range(NCT):
                        nc.tensor.transpose(
                            ptr[:, ct * 128:(ct + 1) * 128],
                            f2t[:, ct, st * 128:(st + 1) * 128], ident_h[:])
                    nc.vector.tensor_add(out=t2[:], in0=ptr[:],
                                         in1=out2_s[:, st, :])
                    t2b = poolB.tile([128, D], f16, tag="t2b", bufs=4)
                    nc.vector.tensor_add(out=t2b[:], in0=t2[:], in1=bvec_s["b2b"][:])
                    oo = poolB.tile([128, D], f32, tag="oo", bufs=2)
                    ln_block(poolB, t2b[:], "g3b", "be3b", oo[:])
                    nc.sync.dma_start(
                        out=out_d[st * 128:(st + 1) * 128, :], in_=oo[:])

    import concourse.mybir as mybir2
    _split_multi_waits(nc, mybir2)
    return nc


def _host_prep(inputs):
    x = np.asarray(inputs["enc_inputs"], dtype=np.float32)        # [B, S, D]
    A = np.asarray(inputs["A"], dtype=np.float32)                 # [D, D]
    Wk = np.asarray(inputs["Wk"], dtype=np.float32)               # [D, HID]
    Wq = np.asarray(inputs["Wq"], dtype=np.float32)
    scale = np.float32(1.0 / np.sqrt(D))
    WKP = np.einsum("chd,de->che", Wk.reshape(D, H, D), A).reshape(D, HID) * scale
    WQP = np.einsum("chd,de->che", Wq.reshape(D, H, D), A).reshape(D, HID)
    b1 = np.asarray(inputs["b1"], dtype=np.float32)
    b2 = np.asarray(inputs["b2"], dtype=np.float32)

    def bcast(v):
        return np.ascontiguousarray(np.broadcast_to(
            np.asarray(v, np.float32).astype(np.float16)[None, :], (128, D)))

    f16 = np.float16
    common = {
        "wkp": np.ascontiguousarray(WKP.astype(f16)),
        "wqpt": np.ascontiguousarray(WQP.T.astype(f16)),
        "wv": np.ascontiguousarray(np.asarray(inputs["Wv"], np.float32).astype(f16)),
        "wo": np.ascontiguousarray(np.asarray(inputs["Wo"], np.float32).astype(f16)),
        "w1": np.ascontiguousarray(np.asarray(inputs["W1"], np.float32).astype(f16)),
        "w2": np.ascontiguousarray(np.asarray(inputs["W2"], np.float32).astype(f16)),
        "b1t": np.ascontiguousarray(b1.reshape(NST, 128).T),
        "b2b": bcast(b2),
        "g1b": bcast(inputs["ln_attn_g"]), "be1b": bcast(inputs["ln_attn_b"]),
        "g2b": bcast(inputs["ln_enc_g"]), "be2b": bcast(inputs["ln_enc_b"]),
        "g3b": bcast(inputs["ln_ffn_g"]), "be3b": bcast(inputs["ln_ffn_b"]),
    }
    in_maps = []
    for b in range(B):
        xb = x[b]
        m = dict(common)
        m["xh"] = np.ascontiguousarray(xb.astype(f16))
        m["xTh"] = np.ascontiguousarray(xb.T.astype(f16))
        in_maps.append(m)
    return in_maps


def kernel(**inputs):
    import os
    from concourse.bass_utils import run_bass_kernel_spmd

    if "nc" not in _CACHE:
        _CACHE["nc"] = _build()
    nc = _CACHE["nc"]
    in_maps = _host_prep(inputs)
    trace = bool(os.environ.get("KERNEL_TRACE"))
    res = run_bass_kernel_spmd(nc, in_maps, list(range(B)), trace=trace,
                               tmpdir=os.environ.get("KERNEL_TRACE_DIR") or None)
    if trace:
        _CACHE["last_result"] = res
        if res.exec_time_ns is not None:
            print(f"HW exec time: {res.exec_time_ns} ns")
    out = np.stack([res.results[b]["out_o"] for b in range(B)])      # [B, S, D]
    attn = np.stack([res.results[b]["attn_o"] for b in range(B)])    # [B, H, D, S]
    return out, attn


# revision 31
# speedup vs baseline: 3.3377x; 1.0302x over previous
"""Trainium2 Bass kernel for nn_EncoderLayer_54116587929733.

Data-parallel over batch: B=8 batches -> 8 NeuronCores, no collectives.

Per-core math (batch b, S=2048, d=256, H=8 heads, hid=2048), per head h:
    scores = A^T K_h^T / sqrt(d)            [d, S]
    attn   = softmax_S(scores)              [d, S]   (output!)
    M      = attn @ V_h                     [d, d]
    QA     = Q_h @ A                        [S, d]
    ctx    = QA @ M^T                       [S, d]
then concat heads, out-proj + 3 LayerNorms + FFN.

Key restructurings (exact algebra):
  * A folded into the K/Q projections on the host:
      WKP_h = Wk_h @ A / sqrt(d)  =>  scores = WKP_h^T x^T  (K never built)
      WQP_h = Wq_h @ A            =>  QA^T   = WQP_h^T x^T  (Q never built)
  * V never materialized:  M^T = Wv_h^T (x^T ETu), with G = x^T ETu
    (S-contraction) first, then a tiny d x d matmul.
  * Softmax denominators ride a ones-vector matmul over ETu; the
    normalization is folded into the scores-exp as a per-partition bias
    exp(s + ln(1/denom)) and into a per-partition scale of WMU (below).
  * The whole Q side is LINEAR in x given the (runtime) attention
    matrices, so it collapses to ONE 256x256 matrix:
      out_attn = x @ W*,   W* = sum_h WQP_h MuT_h diag(1/denom) Wo_h
    computed on-chip from tiny d x d matmuls.
  * Heads processed in pairs so the scores^T/G matmuls run at N=512.
  * Out-proj and FFN2 run transposed ([c, s] out, N=512) and transpose
    back via PE-transpose blocks fused into the residual adds.

Everything runs in fp16 (fp32 PSUM accumulation): ~1e-3 accuracy at
full PE speed with fast-weight-load.  All value magnitudes were checked
to sit inside fp16 normal range.
"""

import sys

sys.path.insert(0, "/opt/trn_rl_repo")

import numpy as np

S = 2048
D = 256
H = 8
B = 8
HID = 2048
EPS = 1e-5
NST = S // 128      # 16 s-tiles
NCT = D // 128      # 2 c/e/dl-tiles
NCH = 4             # s-chunks of 512
CH = S // NCH       # 512

_CACHE = {}


def _split_multi_waits(nc, mybir):
    """This walrus build supports only ONE sync-wait per instruction;
    Tile attaches several.  Move extras onto same-engine NOPs inserted
    just before the offending instruction."""
    cnt = 0
    for fn in nc.m.functions:
        for blk in fn.blocks:
            new_insts = []
            for inst in blk.instructions:
                si = inst.sync_info
                if si is not None and si.on_wait and len(si.on_wait) > 1:
                    waits = list(si.on_wait)
                    for w in waits[:-1]:
                        cnt += 1
                        new_insts.append(mybir.InstNoOp(
                            name=f"{inst.name}_wsplit{cnt}", ins=[], outs=[],
                            engine=inst.engine,
                            sync_info=mybir.SyncInfo(on_wait=[w], on_update=[])))
                    si.on_wait = waits[-1:]
                new_insts.append(inst)
            blk.instructions = new_insts
    return cnt


def _build():
    import concourse.bass as bass
    import concourse.mybir as mybir
    import concourse.tile as tile
    from concourse.masks import make_identity

    f32 = mybir.dt.float32
    f16 = mybir.dt.float16
    AF = mybir.ActivationFunctionType

    nc = bass.Bass(target_bir_lowering=False)

    # ---- DRAM I/O (fp16 operands, fp32 outputs) --------------------
    xh_d = nc.dram_tensor("xh", [S, D], f16, kind="ExternalInput")
    xTh_d = nc.dram_tensor("xTh", [D, S], f16, kind="ExternalInput")
    wkp_d = nc.dram_tensor("wkp", [D, HID], f16, kind="ExternalInput")
    wv_d = nc.dram_tensor("wv", [D, HID], f16, kind="ExternalInput")
    wqpt_d = nc.dram_tensor("wqpt", [HID, D], f16, kind="ExternalInput")
    wo_d = nc.dram_tensor("wo", [HID, D], f16, kind="ExternalInput")
    w1_d = nc.dram_tensor("w1", [D, HID], f16, kind="ExternalInput")
    w2_d = nc.dram_tensor("w2", [HID, D], f16, kind="ExternalInput")
    b1t_d = nc.dram_tensor("b1t", [128, NST], f32, kind="ExternalInput")
    bvec_names = ["b2b", "g1b", "be1b", "g2b", "be2b", "g3b", "be3b"]
    bvec_d = {n: nc.dram_tensor(n, [128, D], f16, kind="ExternalInput")
              for n in bvec_names}

    attn_d = nc.dram_tensor("attn_o", [H, D, S], f32, kind="ExternalOutput")
    out_d = nc.dram_tensor("out_o", [S, D], f32, kind="ExternalOutput")

    with tile.TileContext(nc) as tc:
        import contextlib
        ctx = contextlib.ExitStack()
        with ctx:
            persist = ctx.enter_context(tc.tile_pool(name="persist", bufs=1))

            # attention-phase operands first (compute starts on these)
            xTh_s = persist.tile([128, NCT, S], f16)
            nc.sync.dma_start(out=xTh_s[:], in_=xTh_d.rearrange("(t p) s -> p t s", p=128))
            xh_s = persist.tile([128, NST, D], f16)
            wqpt_s = persist.tile([128, NST, D], f16)
            wo_s = persist.tile([128, NST, D], f16)
            w1_s = persist.tile([128, NCT, HID], f16)
            nc.scalar.dma_start(out=w1_s[:], in_=w1_d.rearrange("(t p) n -> p t n", p=128))
            w2_s = persist.tile([128, NST, D], f16)
            nc.scalar.dma_start(out=w2_s[:], in_=w2_d.rearrange("(t p) c -> p t c", p=128))
            b1t_s = persist.tile([128, NST], f32)
            nc.scalar.dma_start(out=b1t_s[:], in_=b1t_d[:])
            bvec_s = {}
            for n in bvec_names:
                bvec_s[n] = persist.tile([128, D], f16, name=f"bv_{n}", tag=f"bv_{n}")
                nc.scalar.dma_start(out=bvec_s[n][:], in_=bvec_d[n][:])

            eps_s = persist.tile([128, 1], f32)
            nc.vector.memset(eps_s, EPS)
            ones_f = persist.tile([128, 1], f32)
            nc.vector.memset(ones_f, 1.0)
            zero_s = persist.tile([128, 1], f32)
            nc.vector.memset(zero_s, 0.0)
            ones_h = persist.tile([128, 1], f16)
            nc.vector.tensor_copy(out=ones_h[:], in_=ones_f[:])
            ident_h = persist.tile([128, 128], f16)
            make_identity(nc, ident_h)

            # A->B carriers
            mut_s = persist.tile([128, NCT, H, D], f16)      # [j, jt, h, dl]
            recip_s = persist.tile([128, H, NCT], f32)       # 1/denom [dl, h, dlt]
            wmut_s = persist.tile([128, NCT, H, D], f16)     # (WQP_h MuT_h)^T * recip
            wst_s = persist.tile([128, NCT, D], f16)         # W*

            # ------------- Phase A: K/V side, head PAIRS -------------
            with (
                tc.tile_pool(name="poolA", bufs=1) as poolA,
                tc.tile_pool(name="psA", bufs=1, space="PSUM") as psA,
            ):
                wkp_s = poolA.tile([128, NCT, HID], f16, tag="wkp")
                wkp_r = wkp_d.rearrange("(t p) n -> p t n", p=128)
                for _pr in range(H // 2):
                    nc.sync.dma_start(
                        out=wkp_s[:, :, _pr * 2 * D:(_pr + 1) * 2 * D],
                        in_=wkp_r[:, :, _pr * 2 * D:(_pr + 1) * 2 * D])
                wv_s = poolA.tile([128, NCT, HID], f16, tag="wv")
                nc.sync.dma_start(out=xh_s[:], in_=xh_d.rearrange("(t p) c -> p t c", p=128))
                nc.sync.dma_start(out=wv_s[:], in_=wv_d.rearrange("(t p) n -> p t n", p=128))
                nc.scalar.dma_start(out=wqpt_s[:], in_=wqpt_d.rearrange("(t p) c -> p t c", p=128))
                nc.scalar.dma_start(out=wo_s[:], in_=wo_d.rearrange("(t p) c -> p t c", p=128))

                for pr in range(H // 2):
                    h0 = 2 * pr
                    ps0 = h0 * D
                    # scores^T for the PAIR: ETu [s, 512] = exp(x WKP_pair)
                    etu_s = poolA.tile([128, NST, 2 * D], f16, tag="etu", bufs=3)
                    for st in range(NST):
                        stp = psA.tile([128, 2 * D], f32, tag="stp", bufs=2)
                        for kt in range(NCT):
                            nc.tensor.matmul(
                                stp[:], xTh_s[:, kt, st * 128:(st + 1) * 128],
                                wkp_s[:, kt, ps0:ps0 + 2 * D],
                                start=(kt == 0), stop=(kt == NCT - 1))
                        nc.scalar.activation(out=etu_s[:, st, :], in_=stp[:],
                                             func=AF.Exp)

                    # pair denominators: [1, 512] = sum_s ETu
                    pd = psA.tile([1, 2 * D], f32, tag="pdx", bufs=1)
                    for st in range(NST):
                        nc.tensor.matmul(pd[:], ones_h[:, :1], etu_s[:, st, :],
                                         start=(st == 0), stop=(st == NST - 1))
                    denr = poolA.tile([1, 2 * D], f32, tag="denr", bufs=1)
                    nc.vector.tensor_copy(out=denr[:], in_=pd[:])
                    for q in range(4):           # q = hh*2 + et
                        hh, et = divmod(q, 2)
                        pdt = psA.tile([128, 1], f32, tag="pdx", bufs=1)
                        nc.tensor.matmul(pdt[:], denr[:1, q * 128:(q + 1) * 128],
                                         ones_f[:1, :1], start=True, stop=True)
                        nc.vector.reciprocal(out=recip_s[:, h0 + hh, et:et + 1],
                                             in_=pdt[:])

                    # G = x^T @ ETu  [c, 512]  (V-side contraction over S)
                    g_s = poolA.tile([128, NCT, 2 * D], f16, tag="g2", bufs=2)
                    for ct in range(NCT):
                        pg = psA.tile([128, 2 * D], f32, tag="pg", bufs=2)
                        for st in range(NST):
                            nc.tensor.matmul(
                                pg[:], xh_s[:, st, ct * 128:(ct + 1) * 128],
                                etu_s[:, st, :],
                                start=(st == 0), stop=(st == NST - 1))
                        nc.vector.tensor_copy(out=g_s[:, ct, :], in_=pg[:])

                    # MuT_h = Wv_h^T @ G_h   [j, dl]
                    for hh in range(2):
                        h = h0 + hh
                        for jt in range(NCT):
                            pmu = psA.tile([128, D], f32, tag="pmu", bufs=1)
                            for ct in range(NCT):
                                nc.tensor.matmul(
                                    pmu[:],
                                    wv_s[:, ct, h * D + jt * 128:h * D + (jt + 1) * 128],
                                    g_s[:, ct, hh * D:(hh + 1) * D],
                                    start=(ct == 0), stop=(ct == NCT - 1))
                            nc.vector.tensor_copy(out=mut_s[:, jt, h, :], in_=pmu[:])
                    # attention output: E = transpose(ETu) * recip
                    for hh in range(2):
                        h = h0 + hh
                        e_s = poolA.tile([128, NCT, S], f16, tag="e", bufs=3)
                        for et in range(NCT):
                            ecol = hh * D + et * 128
                            for sb in range(2):
                                pte = psA.tile([128, 8 * 128], f16, tag="pte",
                                               bufs=2)
                                for st8 in range(8):
                                    st = sb * 8 + st8
                                    nc.tensor.transpose(
                                        pte[:, st8 * 128:(st8 + 1) * 128],
                                        etu_s[:, st, ecol:ecol + 128],
                                        ident_h[:])
                                nc.vector.tensor_scalar_mul(
                                    out=e_s[:, et, sb * 1024:(sb + 1) * 1024],
                                    in0=pte[:],
                                    scalar1=recip_s[:, h, et:et + 1])
                            nc.gpsimd.dma_start(
                                out=attn_d[h, et * 128:(et + 1) * 128, :],
                                in_=e_s[:, et, :])


            # Collapse the whole Q side:  out_attn = x @ W*  with
            #   W* = sum_h WQP_h MuT_h diag(recip) Wo_h   (256 x 256, runtime)
            # recip is folded into the WMUT evacuation (per-partition scale).
            with tc.tile_pool(name="psW", bufs=1, space="PSUM") as psW:
                for h in range(H):
                    for dlt in range(NCT):
                        pwm = psW.tile([128, D], f32, tag="pwm", bufs=2)
                        for et in range(NCT):
                            nc.tensor.matmul(
                                pwm[:],
                                mut_s[:, et, h, dlt * 128:(dlt + 1) * 128],
                                wqpt_s[:, h * NCT + et, :],
                                start=(et == 0), stop=(et == NCT - 1))
                        nc.vector.tensor_scalar_mul(
                            out=wmut_s[:, dlt, h, :], in0=pwm[:],
                            scalar1=recip_s[:, h, dlt:dlt + 1])
                for ct in range(NCT):
                    pws = psW.tile([128, D], f32, tag="pws", bufs=2)
                    nkt = H * NCT
                    for h in range(H):
                        for dlt in range(NCT):
                            ki = h * NCT + dlt
                            nc.tensor.matmul(
                                pws[:],
                                wmut_s[:, dlt, h, ct * 128:(ct + 1) * 128],
                                wo_s[:, h * NCT + dlt, :],
                                start=(ki == 0), stop=(ki == nkt - 1))
                    nc.vector.tensor_copy(out=wst_s[:, ct, :], in_=pws[:])

            # ---------- Phase B: out-proj + FFN, per s-chunk ----------
            def ln_block(pool, src_ap, gname, bname, dst_ap):
                """dst = LN(src) * g + b   (fp16 chain, stats fp32)"""
                stats = pool.tile([128, 6], f32, tag="lnstats", bufs=8)
                nc.vector.bn_stats(out=stats[:], in_=src_ap)
                mv = pool.tile([128, 2], f32, tag="lnmv", bufs=8)
                nc.vector.bn_aggr(out=mv[:], in_=stats[:])
                nc.scalar.activation(out=mv[:, 1:2], in_=mv[:, 1:2],
                                     func=AF.Sqrt, bias=eps_s[:], scale=1.0)
                nc.vector.reciprocal(out=mv[:, 1:2], in_=mv[:, 1:2])
                y = pool.tile([128, D], f16, tag="lny", bufs=6)
                nc.vector.tensor_scalar(
                    out=y[:], in0=src_ap,
                    scalar1=mv[:, 0:1], scalar2=mv[:, 1:2],
                    op0=mybir.AluOpType.subtract, op1=mybir.AluOpType.mult)
                nc.vector.tensor_mul(out=y[:], in0=y[:], in1=bvec_s[gname][:])
                nc.vector.tensor_add(out=dst_ap, in0=y[:], in1=bvec_s[bname][:])

            with (
                tc.tile_pool(name="poolB", bufs=1) as poolB,
                tc.tile_pool(name="psB", bufs=1, space="PSUM") as psB,
            ):
                # B1: out-proj  woT [c', s-chunk] = W*^T x^T (per-chunk tiles
                # keep the dependency graph chunk-granular)
                wot_c = []
                for chk in range(NCH):
                    c0 = chk * CH
                    wot = poolB.tile([128, NCT, CH], f16, tag="wot", bufs=4,
                                     name=f"wot{chk}")
                    wot_c.append(wot)
                    for ct in range(NCT):
                        pwo = psB.tile([128, CH], f32, tag="pwo", bufs=2)
                        for kt in range(NCT):
                            nc.tensor.matmul(
                                pwo[:], wst_s[:, kt, ct * 128:(ct + 1) * 128],
                                xTh_s[:, kt, c0:c0 + CH],
                                start=(kt == 0), stop=(kt == NCT - 1))
                        nc.scalar.activation(out=wot[:, ct, :],
                                             in_=pwo[:], func=AF.Copy)

                # B2: transpose back + residual + LN1 + LN2 (per s-tile)
                out2_c = []
                for chk in range(NCH):
                    out2 = poolB.tile([128, NCH, D], f16, tag="out2", bufs=4,
                                      name=f"out2_{chk}")
                    out2_c.append(out2)
                    for sti in range(NCH):
                        st = chk * NCH + sti
                        t1 = poolB.tile([128, D], f16, tag="t1", bufs=4)
                        ptr = psB.tile([128, D], f16, tag="ptr", bufs=3)
                        for ct in range(NCT):
                            nc.tensor.transpose(
                                ptr[:, ct * 128:(ct + 1) * 128],
                                wot_c[chk][:, ct, sti * 128:(sti + 1) * 128],
                                ident_h[:])
                        nc.vector.tensor_add(out=t1[:], in0=ptr[:],
                                             in1=xh_s[:, st, :])
                        o1 = poolB.tile([128, D], f16, tag="o1", bufs=4)
                        ln_block(poolB, t1[:], "g1b", "be1b", o1[:])
                        t1b = poolB.tile([128, D], f16, tag="t1b", bufs=4)
                        nc.vector.tensor_add(out=t1b[:], in0=o1[:],
                                             in1=xh_s[:, st, :])
                        ln_block(poolB, t1b[:], "g2b", "be2b", out2[:, sti, :])

                # B3: transpose out2 -> [c, s-chunk] for FFN1
                o2t_c = []
                for chk in range(NCH):
                    o2t = poolB.tile([128, NCT, CH], f16, tag="o2t", bufs=4,
                                     name=f"o2t{chk}")
                    o2t_c.append(o2t)
                    for sti in range(NCH):
                        ptr = psB.tile([128, D], f16, tag="ptr", bufs=3)
                        for ct in range(NCT):
                            nc.tensor.transpose(
                                ptr[:, ct * 128:(ct + 1) * 128],
                                out2_c[chk][:, sti, ct * 128:(ct + 1) * 128],
                                ident_h[:])
                        for ct in range(NCT):
                            nc.scalar.activation(
                                out=o2t[:, ct, sti * 128:(sti + 1) * 128],
                                in_=ptr[:, ct * 128:(ct + 1) * 128], func=AF.Copy)

                # B4/B5: FFN per s-chunk (FFN2 of chunk k overlaps FFN1 of k+1)
                f2t_c = []
                for chk in range(NCH):
                    c0 = chk * CH
                    h1c = poolB.tile([128, NST, CH], f16, tag="h1", bufs=2)
                    for jt in range(NST):
                        ph1 = psB.tile([128, CH], f32, tag="ph1", bufs=2)
                        for kt in range(NCT):
                            nc.tensor.matmul(
                                ph1[:], w1_s[:, kt, jt * 128:(jt + 1) * 128],
                                o2t_c[chk][:, kt, :],
                                start=(kt == 0), stop=(kt == NCT - 1))
                        if jt % 2 == 0:
                            nc.scalar.activation(out=h1c[:, jt, :], in_=ph1[:],
                                                 func=AF.Relu,
                                                 bias=b1t_s[:, jt:jt + 1],
                                                 scale=1.0)
                        else:
                            nc.vector.tensor_scalar(
                                out=h1c[:, jt, :], in0=ph1[:],
                                scalar1=b1t_s[:, jt:jt + 1], scalar2=zero_s[:],
                                op0=mybir.AluOpType.add,
                                op1=mybir.AluOpType.max)
                    f2t = poolB.tile([128, NCT, CH], f16, tag="f2t", bufs=4,
                                     name=f"f2t{chk}")
                    f2t_c.append(f2t)
                    for ct in range(NCT):
                        pf2 = psB.tile([128, CH], f32, tag="pf2", bufs=1)
                        for jt in range(NST):
                            nc.tensor.matmul(
                                pf2[:], w2_s[:, jt, ct * 128:(ct + 1) * 128],
                                h1c[:, jt, :],
                                start=(jt == 0), stop=(jt == NST - 1))
                        nc.scalar.activation(out=f2t[:, ct, :],
                                             in_=pf2[:], func=AF.Copy)

                # B6: transpose back + bias + residual + LN3 -> DMA
                out_r = out_d.rearrange("(t p) c -> p t c", p=128)
                for chk in range(NCH):
                    oo = poolB.tile([128, NCH, D], f32, tag="oo", bufs=2,
                                    name=f"oo{chk}")
                    for sti in range(NCH):
                        st = chk * NCH + sti
                        t2 = poolB.tile([128, D], f16, tag="t2", bufs=4)
                        ptr = psB.tile([128, D], f16, tag="ptr", bufs=3)
                        for ct in range(NCT):
                            nc.tensor.transpose(
                                ptr[:, ct * 128:(ct + 1) * 128],
                                f2t_c[chk][:, ct, sti * 128:(sti + 1) * 128],
                                ident_h[:])
                        nc.vector.tensor_add(out=t2[:], in0=ptr[:],
                                             in1=out2_c[chk][:, sti, :])
                        t2b = poolB.tile([128, D], f16, tag="t2b", bufs=4)
                        nc.vector.tensor_add(out=t2b[:], in0=t2[:],
                                             in1=bvec_s["b2b"][:])
                        ln_block(poolB, t2b[:], "g3b", "be3b", oo[:, sti, :])
                    nc.sync.dma_start(
                        out=out_r[:, chk * NCH:(chk + 1) * NCH, :], in_=oo[:])

    import concourse.mybir as mybir2
    _split_multi_waits(nc, mybir2)
    return nc


def _host_prep(inputs):
    x = np.asarray(inputs["enc_inputs"], dtype=np.float32)        # [B, S, D]
    A = np.asarray(inputs["A"], dtype=np.float32)                 # [D, D]
    Wk = np.asarray(inputs["Wk"], dtype=np.float32)               # [D, HID]
    Wq = np.asarray(inputs["Wq"], dtype=np.float32)
    scale = np.float32(1.0 / np.sqrt(D))
    WKP = np.einsum("chd,de->che", Wk.reshape(D, H, D), A).reshape(D, HID) * scale
    WQP = np.einsum("chd,de->che", Wq.reshape(D, H, D), A).reshape(D, HID)
    b1 = np.asarray(inputs["b1"], dtype=np.float32)
    b2 = np.asarray(inputs["b2"], dtype=np.float32)

    def bcast(v):
        return np.ascontiguousarray(np.broadcast_to(
            np.asarray(v, np.float32).astype(np.float16)[None, :], (128, D)))

    f16 = np.float16
    common = {
        "wkp": np.ascontiguousarray(WKP.astype(f16)),
        "wqpt": np.ascontiguousarray(WQP.T.astype(f16)),
        "wv": np.ascontiguousarray(np.asarray(inputs["Wv"], np.float32).astype(f16)),
        "wo": np.ascontiguousarray(np.asarray(inputs["Wo"], np.float32).astype(f16)),
        "w1": np.ascontiguousarray(np.asarray(inputs["W1"], np.float32).astype(f16)),
        "w2": np.ascontiguousarray(np.asarray(inputs["W2"], np.float32).astype(f16)),
        "b1t": np.ascontiguousarray(b1.reshape(NST, 128).T),
        "b2b": bcast(b2),
        "g1b": bcast(inputs["ln_attn_g"]), "be1b": bcast(inputs["ln_attn_b"]),
        "g2b": bcast(inputs["ln_enc_g"]), "be2b": bcast(inputs["ln_enc_b"]),
        "g3b": bcast(inputs["ln_ffn_g"]), "be3b": bcast(inputs["ln_ffn_b"]),
    }
    in_maps = []
    for b in range(B):
        xb = x[b]
        m = dict(common)
        m["xh"] = np.ascontiguousarray(xb.astype(f16))
        m["xTh"] = np.ascontiguousarray(xb.T.astype(f16))
        in_maps.append(m)
    return in_maps


def kernel(**inputs):
    import os
    from concourse.bass_utils import run_bass_kernel_spmd

    if "nc" not in _CACHE:
        _CACHE["nc"] = _build()
    nc = _CACHE["nc"]
    in_maps = _host_prep(inputs)
    trace = bool(os.environ.get("KERNEL_TRACE"))
    res = run_bass_kernel_spmd(nc, in_maps, list(range(B)), trace=trace,
                               tmpdir=os.environ.get("KERNEL_TRACE_DIR") or None)
    if trace:
        _CACHE["last_result"] = res
        if res.exec_time_ns is not None:
            print(f"HW exec time: {res.exec_time_ns} ns")
    out = np.stack([res.results[b]["out_o"] for b in range(B)])      # [B, S, D]
    attn = np.stack([res.results[b]["attn_o"] for b in range(B)])    # [B, H, D, S]
    return out, attn
